# revision 33
# baseline (speedup 1.0000x reference)
"""Multi-head attention (B=2, S=2048, D=1024, H=16) on 8 TRN2 NeuronCores.

Sharding: core c handles batch b = c//4 and head group hg = c%4 (4 heads,
256 features f0 = hg*256). Each core computes Q/K/V projections for its
feature slice, attention for its 4 heads, and a partial output projection
y_partial = attnout @ Wo[:, f0:f0+256].T (emitted in fp16). Host sums the 4
partials per batch and adds bo.

Single fused software-pipelined loop, engineered so the tensor engine's
instruction stream is dense (TRN2 PE DVFS only reaches 2.4GHz after ~3us of
continuous execution):
 - K/V projections are JIT-streamed per k-tile-group inside q-chunk 0's
   attention loop; Q projections prefetched one (qc,pr) iteration ahead;
   out-projection of chunk qc deferred into chunk qc+1's loop. The PE
   therefore always has independent matmuls queued while softmax exp runs.
 - scores per head pair are issued back-to-back as K=64 matmuls on PE row
   groups 0:63 / 64:127 (tile_position auto-derived from base partition) so
   the two matmuls execute concurrently on the array.
 - softmax exp: scalar-engine Exp on [128, 2, 512] PSUM spans (two heads per
   instruction), with a fraction of k-tiles offloaded to DVE/Pool via a
   one-instruction fp16 Schraudolph exp (t = round(A*s + B) written as int16,
   bit-reinterpreted as fp16; constant-offset error is common-mode across k
   and cancels in the softmax normalization).
 - normalization: row sums ride along in the AV matmul (ones column in the
   augmented V); psav is drained early to SBUF (frees the PSUM bank), sums
   broadcast across partitions via two K=1 ones-matmuls into one PSUM tile,
   a single reciprocal_approx_fast over all 128 partitions, then two
   multiplies produce attnout.T in fp16.

All matmuls fp16 (PSUM accumulation fp32); elementwise fp32 on DVE/Pool.
"""
import numpy as np

import concourse.bass as bass
import concourse.mybir as mybir
import concourse.tile as tile
from concourse import bacc
from concourse import bass_utils

F32 = mybir.dt.float32
F16 = mybir.dt.float16
I16 = mybir.dt.int16
EXP = mybir.ActivationFunctionType.Exp
ADD = mybir.AluOpType.add
MULT = mybir.AluOpType.mult

B, S, D, H = 2, 2048, 1024, 16
HD = D // H          # 64
E = 256              # local features per core (4 heads)
QC = 512             # q-chunk size for the attention phase
N_QC = S // QC       # 4
N_KT = S // 128      # 16 k-tiles
KD = D // 128        # 8 contraction tiles for the projections

# Schraudolph fp16 exp: bits = round(x * 1024/ln2 + 15360 + C); the 0.125
# logit scale is folded into A. C=-44 minimizes max rel err (~3.1%); the
# constant-offset component cancels in the softmax normalization.
SCHR_A = 0.125 * 1024.0 / float(np.log(2.0))
SCHR_B = 15360.0 - 44.0
# k-tiles offloaded off the scalar engine per (qc, pr) iteration for qc >= 1
# (alternating DVE / Pool).
OFF_DVE = (3, 7, 11, 13)
OFF_POOL = ()


def build():
    nc = bacc.Bacc("TRN2", target_bir_lowering=False, debug=False, num_devices=8)

    xT = nc.dram_tensor("xT", [D, S], F16, kind="ExternalInput").ap()
    # wkq = [wkT | wqT] concatenated host-side so one DMA per chunk loads both
    wkq = nc.dram_tensor("wkq", [D, 2 * E], F16, kind="ExternalInput").ap()
    wvT = nc.dram_tensor("wvT", [D, E], F16, kind="ExternalInput").ap()
    woT = nc.dram_tensor("woT", [E, D], F16, kind="ExternalInput").ap()
    bq2 = nc.dram_tensor("bq2", [128, 2], F32, kind="ExternalInput").ap()
    bk2 = nc.dram_tensor("bk2", [128, 2], F32, kind="ExternalInput").ap()
    bvb = nc.dram_tensor("bvb", [128, E], F32, kind="ExternalInput").ap()
    # full V_aug constant patterns (ones/zeros; V columns overwritten by the
    # V projection): even = ones at col 64, odd = ones at col 0
    vce = nc.dram_tensor("vce", [128, N_KT, 128], F16, kind="ExternalInput").ap()
    vco = nc.dram_tensor("vco", [128, N_KT, 128], F16, kind="ExternalInput").ap()
    ones128 = nc.dram_tensor("ones128", [1, 128], F16, kind="ExternalInput").ap()

    y = nc.dram_tensor("y", [S, D], F16, kind="ExternalOutput").ap()

    with tile.TileContext(nc) as tc:
        with (
            tc.tile_pool(name="pool", bufs=1) as pp,
            tc.tile_pool(name="pexp_pool", bufs=8) as pxp,
            tc.tile_pool(name="work", bufs=4) as wk,
            tc.tile_pool(name="rpool", bufs=2) as rp,
            tc.tile_pool(name="ypool", bufs=4) as yp,
            tc.tile_pool(name="ps_s", bufs=2, space="PSUM") as ps_s,
            tc.tile_pool(name="ps_av", bufs=2, space="PSUM") as ps_av,
            tc.tile_pool(name="ps_misc", bufs=2, space="PSUM") as ps_misc,
        ):
            # ---------------- persistent tiles ----------------
            woT_sb = pp.tile([128, 2, D], F16)
            bvb_sb = pp.tile([128, E], F32)
            ones_sb = pp.tile([128, 128], F16)     # partitions 0 and 64 used
            bq_sb = pp.tile([128, 2], F32)
            bk_sb = pp.tile([128, 2], F32)
            QT_sb = pp.tile([128, 2, S], F16)
            KT_sb = pp.tile([128, 2, S], F16)
            OT_sb = pp.tile([128, 2, S], F16)
            xT_sb = pp.tile([128, KD, S], F16)
            wkq_sb = pp.tile([128, KD, 2 * E], F16)
            wv_sb = pp.tile([128, KD, E], F16)
            # V_aug per pair (128 cols each so the matmul dst is a full
            # 128-partition AP):
            #   even head: [*, kt, 0:64]=V, col 64=1, cols 65:128=0
            #   odd head:  col 0=1, cols 1:64=0, [*, kt, 64:128]=V
            Ve_sb = [pp.tile([128, N_KT, 128], F16, name=f"ve{p}", tag=f"ve{p}")
                     for p in range(2)]
            Vo_sb = [pp.tile([128, N_KT, 128], F16, name=f"vo{p}", tag=f"vo{p}")
                     for p in range(2)]

            # ---------------- input DMAs ----------------
            # All transfers are 2-D [128, X] per-chunk DMAs (the fast DMA
            # path). The warmup is DMA-issue-rate-bound (~0.7us per issue per
            # queue), so K and Q weights are fused into one tensor (wkq) and
            # xT s-block pairs are fused into [128, 1024] transfers.
            # sync: xT blocks 0-1; scalar: wkq then xT blocks 2-3;
            # gpsimd: wv + V_aug constants + woT.
            for k in range(KD):
                nc.sync.dma_start(
                    xT_sb[:, k, 0:1024], xT[k * 128:(k + 1) * 128, 0:1024])
            for k in range(KD):
                nc.scalar.dma_start(
                    wkq_sb[:, k, :], wkq[k * 128:(k + 1) * 128, :])
            nc.scalar.dma_start(bk_sb[:], bk2)
            nc.scalar.dma_start(bq_sb[:], bq2)
            nc.scalar.dma_start(ones_sb[0:1, :], ones128)
            nc.scalar.dma_start(ones_sb[64:65, :], ones128)
            for k in range(KD):
                nc.scalar.dma_start(
                    xT_sb[:, k, 1024:2048], xT[k * 128:(k + 1) * 128, 1024:2048])
            for k in range(KD):
                nc.gpsimd.dma_start(wv_sb[:, k, :], wvT[k * 128:(k + 1) * 128, :])
            for pr in range(2):
                nc.gpsimd.dma_start(Ve_sb[pr][:], vce)
                nc.gpsimd.dma_start(Vo_sb[pr][:], vco)
            nc.gpsimd.dma_start(bvb_sb[:], bvb)
            for p in range(2):
                nc.gpsimd.dma_start(woT_sb[:, p, :], woT[p * 128:(p + 1) * 128, :])

            # ---------------- emit helpers ----------------
            def proj_chain(w0, b_sb, out_sb, ch, g):
                """[128, 512] projection chunk: out_sb[:, ch, g*512:...] =
                W_ch.T @ xT[:, g-block] + b (drained on DVE). w0 selects the
                K (0) or Q (E) half of the fused wkq weights."""
                ssl = slice(g * 512, (g + 1) * 512)
                ps = ps_misc.tile([128, 512], F32, tag="misc")
                for k in range(KD):
                    nc.tensor.matmul(
                        ps[:],
                        wkq_sb[:, k, w0 + ch * 128:w0 + (ch + 1) * 128],
                        xT_sb[:, k, ssl],
                        start=(k == 0), stop=(k == KD - 1))
                nc.vector.tensor_scalar(
                    out_sb[:, ch, ssl], ps[:], b_sb[:, ch:ch + 1], None, ADD)

            def kproj(ch, g):
                proj_chain(0, bk_sb, KT_sb, ch, g)

            def qproj(qc, ch):
                proj_chain(E, bq_sb, QT_sb, ch, qc)

            def vproj(st):
                """V for s-tile st (all 4 heads), scattered+biased into the
                augmented V tiles on Pool."""
                ps = ps_misc.tile([128, 512], F32, tag="misc")
                for k in range(KD):
                    nc.tensor.matmul(
                        ps[:, 0:E],
                        xT_sb[:, k, st * 128:(st + 1) * 128],
                        wv_sb[:, k, :],
                        start=(k == 0), stop=(k == KD - 1))
                for h in range(4):
                    pr, odd = h // 2, h % 2
                    dst = (Vo_sb[pr][:, st, 64:128] if odd
                           else Ve_sb[pr][:, st, 0:64])
                    nc.vector.tensor_tensor(
                        dst, ps[:, h * 64:(h + 1) * 64],
                        bvb_sb[:, h * 64:(h + 1) * 64], ADD)

            def outproj_unit(st, nch):
                """y[s-tile st, nch*512:...] = OT[:, :, ssl].T @ woT (both
                contraction chunks), drained to fp16 on Pool, DMA'd on sync."""
                ssl = slice(st * 128, (st + 1) * 128)
                psy = ps_misc.tile([128, 512], F32, tag="misc")
                for cc in range(2):
                    nc.tensor.matmul(
                        psy[:], OT_sb[:, cc, ssl],
                        woT_sb[:, cc, nch * 512:(nch + 1) * 512],
                        start=(cc == 0), stop=(cc == 1))
                y_sb = yp.tile([128, 512], F16, tag="y")
                nc.vector.tensor_copy(y_sb[:], psy[:])
                eng = nc.sync if nch == 0 else nc.gpsimd
                eng.dma_start(y[ssl, nch * 512:(nch + 1) * 512], y_sb[:])

            def attn_iter(qc, pr, fills):
                """One (q-chunk, head-pair) attention iteration. `fills` maps
                kt -> list of emit callables sprinkled into the loop to keep
                the PE stream dense. AV matmuls run AV_LAG k-tiles behind the
                score matmuls so the PE never waits on the exp latency.
                Returns a callable that emits the PE/DVE/Pool back half of the
                normalization (scheduled as a fill in the next iteration)."""
                qsl = slice(qc * QC, (qc + 1) * QC)
                av_e = ps_av.tile([128, QC], F32, tag="av")
                av_o = ps_av.tile([128, QC], F32, tag="av")
                pexps = {}
                AV_LAG = 3

                def emit_av(kt):
                    pexp = pexps.pop(kt)
                    nc.tensor.matmul(av_e[:], Ve_sb[pr][:, kt, :],
                                     pexp[:, 0, :],
                                     start=(kt == 0), stop=(kt == N_KT - 1))
                    nc.tensor.matmul(av_o[:], Vo_sb[pr][:, kt, :],
                                     pexp[:, 1, :],
                                     start=(kt == 0), stop=(kt == N_KT - 1))

                for kt in range(N_KT):
                    for f in fills.get(kt, ()):
                        f()
                    ksl = slice(kt * 128, (kt + 1) * 128)
                    # scores for the head pair: two K=64 matmuls on PE row
                    # groups 0:63 / 64:127, issued back-to-back so they run
                    # concurrently on the array.
                    ps = ps_s.tile([128, 2, QC], F32, tag="s")
                    nc.tensor.matmul(ps[:, 0, :], KT_sb[0:64, pr, ksl],
                                     QT_sb[0:64, pr, qsl])
                    nc.tensor.matmul(ps[:, 1, :], KT_sb[64:128, pr, ksl],
                                     QT_sb[64:128, pr, qsl])
                    pexp = pxp.tile([128, 2, QC], F16, tag="pexp")
                    pexps[kt] = pexp
                    if qc > 0 and kt in OFF_DVE:
                        nc.vector.tensor_scalar(
                            pexp[:].bitcast(I16), ps[:], SCHR_A, SCHR_B,
                            MULT, ADD)
                    else:
                        nc.scalar.activation(pexp[:], ps[:], EXP, scale=0.125)
                    if kt >= AV_LAG:
                        emit_av(kt - AV_LAG)
                for kt in range(N_KT - AV_LAG, N_KT):
                    emit_av(kt)
                # normalization, front half (DVE): sums rows to SBUF fp16
                # first (unblocks the broadcast matmuls), then drain psav to
                # SBUF (frees the PSUM banks for the next iteration).
                sums16 = wk.tile([128, QC], F16, tag="sums16")
                nc.vector.tensor_copy(sums16[64:65, :], av_e[64:65, :])
                nc.vector.tensor_copy(sums16[0:1, :], av_o[0:1, :])

                def finish_norm(fast=False):
                    # broadcast the raw sums (at partition 64 for even / 0
                    # for odd) via two K=1 ones-matmuls into one PSUM tile,
                    # one approx reciprocal over all 128 partitions, then
                    # scale on Pool (all-SBUF operands).
                    psbc = ps_misc.tile([128, 512], F32, tag="misc")
                    nc.tensor.matmul(psbc[0:64, :], ones_sb[64:65, 0:64],
                                     sums16[64:65, :])
                    nc.tensor.matmul(psbc[64:128, :], ones_sb[0:1, 64:128],
                                     sums16[0:1, :])
                    rec = rp.tile([128, QC], F32, tag="rec")
                    nc.vector.reciprocal_approx_fast(rec[:], psbc[:])
                    nc.vector.tensor_tensor(
                        OT_sb[0:64, pr, qsl], av_e[0:64, :], rec[0:64, :],
                        MULT)
                    nc.vector.tensor_tensor(
                        OT_sb[64:128, pr, qsl], av_o[64:128, :],
                        rec[64:128, :], MULT)

                return finish_norm

            # ---------------- fused main loop ----------------
            # Warmup: K/Q first (their weights lead the scalar queue; scores
            # can then start early, warming up the scalar engine), V after
            # (its weights stream in on the gpsimd queue meanwhile).
            kproj(0, 0)
            qproj(0, 0)
            vproj(0)
            vproj(1)
            kproj(0, 1)
            vproj(2)
            vproj(3)
            vproj(4)
            vproj(5)

            pending_norm = None
            for qc in range(N_QC):
                for pr in range(2):
                    fills = {}
                    if pending_norm is not None:
                        fills.setdefault(1, []).append(pending_norm)
                    if qc == 0:
                        # JIT K/V projections. K chunk `pr` group g must
                        # precede scores kt=4g; V tile st must precede
                        # AV kt=st (prefetch distance 2 + AV lag).
                        if pr == 0:
                            for g in range(2, 4):
                                fills.setdefault(4 * (g - 1) + 2, []).append(
                                    (lambda g=g: kproj(0, g)))
                            for st in range(6, N_KT):
                                fills.setdefault(st - 6, []).append(
                                    (lambda st=st: vproj(st)))
                            fills.setdefault(12, []).append(lambda: qproj(0, 1))
                            fills.setdefault(13, []).append(lambda: kproj(1, 0))
                        else:
                            for g in range(1, 4):
                                fills.setdefault(4 * (g - 1) + 2, []).append(
                                    (lambda g=g: kproj(1, g)))
                            fills.setdefault(12, []).append(lambda: qproj(1, 0))
                    else:
                        # out-projection of the previous q-chunk: 8 units
                        # spread across the two pr iterations.
                        for i, slot in enumerate((6, 9, 12, 15)):
                            u = pr * 4 + i
                            st, nch = (qc - 1) * 4 + u // 2, u % 2
                            fills.setdefault(slot, []).append(
                                (lambda st=st, nch=nch: outproj_unit(st, nch)))
                        if pr == 0:
                            fills.setdefault(12, []).append(
                                (lambda qc=qc: qproj(qc, 1)))
                        elif qc < N_QC - 1:
                            fills.setdefault(12, []).append(
                                (lambda qc=qc: qproj(qc + 1, 0)))
                    pending_norm = attn_iter(qc, pr, fills)
            pending_norm(fast=True)

            # tail: out-projection of the last q-chunk
            for u in range(8):
                st, nch = (N_QC - 1) * 4 + u // 2, u % 2
                outproj_unit(st, nch)

    nc.compile()
    return nc


_NC_CACHE = None
last_in_maps = None


def kernel(x, Wq, bq, Wk, bk, Wv, bv, Wo, bo):
    global _NC_CACHE, last_in_maps
    x = np.asarray(x, dtype=np.float32)
    Wq, bq = np.asarray(Wq, np.float32), np.asarray(bq, np.float32)
    Wk, bk = np.asarray(Wk, np.float32), np.asarray(bk, np.float32)
    Wv, bv = np.asarray(Wv, np.float32), np.asarray(bv, np.float32)
    Wo, bo = np.asarray(Wo, np.float32), np.asarray(bo, np.float32)

    if _NC_CACHE is None:
        _NC_CACHE = build()
    nc = _NC_CACHE

    vce = np.zeros((128, N_KT, 128), np.float16)
    vce[:, :, 64] = 1.0
    vco = np.zeros((128, N_KT, 128), np.float16)
    vco[:, :, 0] = 1.0
    ones128 = np.ones((1, 128), np.float16)

    in_maps = []
    for c in range(8):
        b, f0 = c // 4, (c % 4) * E
        fs = slice(f0, f0 + E)
        in_maps.append(dict(
            xT=np.ascontiguousarray(x[b].T).astype(np.float16),
            wkq=np.ascontiguousarray(np.concatenate(
                [Wk[fs, :].T, Wq[fs, :].T], axis=1)).astype(np.float16),
            wvT=np.ascontiguousarray(Wv[fs, :].T).astype(np.float16),
            woT=np.ascontiguousarray(Wo[:, fs].T).astype(np.float16),
            bq2=np.ascontiguousarray(bq[fs].reshape(2, 128).T),
            bk2=np.ascontiguousarray(bk[fs].reshape(2, 128).T),
            bvb=np.ascontiguousarray(np.broadcast_to(bv[fs], (128, E))),
            vce=vce,
            vco=vco,
            ones128=ones128,
        ))

    last_in_maps = in_maps
    res = bass_utils.run_bass_kernel_spmd(nc, in_maps, core_ids=list(range(8)))

    out = np.zeros((B, S, D), np.float32)
    for c in range(8):
        out[c // 4] += res.results[c]["y"].astype(np.float32)
    out += bo
    return out


# revision 35
# speedup vs baseline: 1.0087x; 1.0087x over previous
"""Multi-head attention (B=2, S=2048, D=1024, H=16) on 8 TRN2 NeuronCores.

Sharding: core c handles batch b = c//4 and head group hg = c%4 (4 heads,
256 features f0 = hg*256). Each core computes Q/K/V projections for its
feature slice, attention for its 4 heads, and a partial output projection
y_partial = attnout @ Wo[:, f0:f0+256].T (emitted in fp16). Host sums the 4
partials per batch and adds bo.

Single fused software-pipelined loop, engineered so the tensor engine's
instruction stream is dense (TRN2 PE DVFS only reaches 2.4GHz after ~3us of
continuous execution):
 - K/V projections are JIT-streamed per k-tile-group inside q-chunk 0's
   attention loop; Q projections prefetched one (qc,pr) iteration ahead;
   out-projection of chunk qc deferred into chunk qc+1's loop. The PE
   therefore always has independent matmuls queued while softmax exp runs.
 - scores per head pair are issued back-to-back as K=64 matmuls on PE row
   groups 0:63 / 64:127 (tile_position auto-derived from base partition) so
   the two matmuls execute concurrently on the array.
 - softmax exp: scalar-engine Exp on [128, 2, 512] PSUM spans (two heads per
   instruction), with a fraction of k-tiles offloaded to DVE/Pool via a
   one-instruction fp16 Schraudolph exp (t = round(A*s + B) written as int16,
   bit-reinterpreted as fp16; constant-offset error is common-mode across k
   and cancels in the softmax normalization).
 - normalization: row sums ride along in the AV matmul (ones column in the
   augmented V); psav is drained early to SBUF (frees the PSUM bank), sums
   broadcast across partitions via two K=1 ones-matmuls into one PSUM tile,
   a single reciprocal_approx_fast over all 128 partitions, then two
   multiplies produce attnout.T in fp16.

All matmuls fp16 (PSUM accumulation fp32); elementwise fp32 on DVE/Pool.
"""
import numpy as np

import concourse.bass as bass
import concourse.mybir as mybir
import concourse.tile as tile
from concourse import bacc
from concourse import bass_utils

F32 = mybir.dt.float32
F16 = mybir.dt.float16
I16 = mybir.dt.int16
EXP = mybir.ActivationFunctionType.Exp
ADD = mybir.AluOpType.add
MULT = mybir.AluOpType.mult

B, S, D, H = 2, 2048, 1024, 16
HD = D // H          # 64
E = 256              # local features per core (4 heads)
QC = 512             # q-chunk size for the attention phase
N_QC = S // QC       # 4
N_KT = S // 128      # 16 k-tiles
KD = D // 128        # 8 contraction tiles for the projections

# Schraudolph fp16 exp: bits = round(x * 1024/ln2 + 15360 + C); the 0.125
# logit scale is folded into A. C=-44 minimizes max rel err (~3.1%); the
# constant-offset component cancels in the softmax normalization.
SCHR_A = 0.125 * 1024.0 / float(np.log(2.0))
SCHR_B = 15360.0 - 44.0
# k-tiles offloaded off the scalar engine per (qc, pr) iteration for qc >= 1
# (alternating DVE / Pool).
OFF_DVE = (3, 7, 10, 13)
OFF_POOL = ()


def build():
    nc = bacc.Bacc("TRN2", target_bir_lowering=False, debug=False, num_devices=8)

    xT = nc.dram_tensor("xT", [D, S], F16, kind="ExternalInput").ap()
    # wkq = [wkT | wqT] concatenated host-side so one DMA per chunk loads both
    wkq = nc.dram_tensor("wkq", [D, 2 * E], F16, kind="ExternalInput").ap()
    wvT = nc.dram_tensor("wvT", [D, E], F16, kind="ExternalInput").ap()
    woT = nc.dram_tensor("woT", [E, D], F16, kind="ExternalInput").ap()
    bq2 = nc.dram_tensor("bq2", [128, 2], F32, kind="ExternalInput").ap()
    bk2 = nc.dram_tensor("bk2", [128, 2], F32, kind="ExternalInput").ap()
    bvb = nc.dram_tensor("bvb", [128, E], F32, kind="ExternalInput").ap()
    # full V_aug constant patterns (ones/zeros; V columns overwritten by the
    # V projection): even = ones at col 64, odd = ones at col 0
    vce = nc.dram_tensor("vce", [128, N_KT, 128], F16, kind="ExternalInput").ap()
    vco = nc.dram_tensor("vco", [128, N_KT, 128], F16, kind="ExternalInput").ap()

    y = nc.dram_tensor("y", [S, D], F16, kind="ExternalOutput").ap()

    with tile.TileContext(nc) as tc:
        with (
            tc.tile_pool(name="pool", bufs=1) as pp,
            tc.tile_pool(name="pexp_pool", bufs=8) as pxp,
            tc.tile_pool(name="work", bufs=4) as wk,
            tc.tile_pool(name="rpool", bufs=2) as rp,
            tc.tile_pool(name="ypool", bufs=4) as yp,
            tc.tile_pool(name="ps_s", bufs=2, space="PSUM") as ps_s,
            tc.tile_pool(name="ps_av", bufs=2, space="PSUM") as ps_av,
            tc.tile_pool(name="ps_misc", bufs=2, space="PSUM") as ps_misc,
        ):
            # ---------------- persistent tiles ----------------
            woT_sb = pp.tile([128, 2, D], F16)
            bvb_sb = pp.tile([128, E], F32)
            ones_sb = pp.tile([128, 128], F16)     # partitions 0 and 64 used
            bq_sb = pp.tile([128, 2], F32)
            bk_sb = pp.tile([128, 2], F32)
            QT_sb = pp.tile([128, 2, S], F16)
            KT_sb = pp.tile([128, 2, S], F16)
            OT_sb = pp.tile([128, 2, S], F16)
            xT_sb = pp.tile([128, KD, S], F16)
            wkq_sb = pp.tile([128, KD, 2 * E], F16)
            wv_sb = pp.tile([128, KD, E], F16)
            # V_aug per pair (128 cols each so the matmul dst is a full
            # 128-partition AP):
            #   even head: [*, kt, 0:64]=V, col 64=1, cols 65:128=0
            #   odd head:  col 0=1, cols 1:64=0, [*, kt, 64:128]=V
            Ve_sb = [pp.tile([128, N_KT, 128], F16, name=f"ve{p}", tag=f"ve{p}")
                     for p in range(2)]
            Vo_sb = [pp.tile([128, N_KT, 128], F16, name=f"vo{p}", tag=f"vo{p}")
                     for p in range(2)]

            # ---------------- input DMAs ----------------
            # All transfers are 2-D [128, X] per-chunk DMAs (the fast DMA
            # path). The warmup is DMA-issue-rate-bound (~0.7us per issue per
            # queue), so K and Q weights are fused into one tensor (wkq) and
            # xT s-block pairs are fused into [128, 1024] transfers.
            # sync: xT blocks 0-1; scalar: wkq then xT blocks 2-3;
            # gpsimd: wv + V_aug constants + woT.
            for k in range(KD):
                nc.sync.dma_start(
                    xT_sb[:, k, 0:1024], xT[k * 128:(k + 1) * 128, 0:1024])
            for k in range(KD):
                nc.scalar.dma_start(
                    wkq_sb[:, k, :], wkq[k * 128:(k + 1) * 128, :])
            nc.scalar.dma_start(bk_sb[:], bk2)
            nc.scalar.dma_start(bq_sb[:], bq2)
            for k in range(KD):
                nc.scalar.dma_start(
                    xT_sb[:, k, 1024:2048], xT[k * 128:(k + 1) * 128, 1024:2048])
            for k in range(KD):
                nc.gpsimd.dma_start(wv_sb[:, k, :], wvT[k * 128:(k + 1) * 128, :])
            for pr in range(2):
                nc.gpsimd.dma_start(Ve_sb[pr][:], vce)
                nc.gpsimd.dma_start(Vo_sb[pr][:], vco)
            nc.gpsimd.dma_start(bvb_sb[:], bvb)
            for p in range(2):
                nc.gpsimd.dma_start(woT_sb[:, p, :], woT[p * 128:(p + 1) * 128, :])

            nc.vector.memset(ones_sb[0:1, :], 1.0)
            nc.vector.memset(ones_sb[64:65, :], 1.0)

            # ---------------- emit helpers ----------------
            def proj_chain(w0, b_sb, out_sb, ch, g):
                """[128, 512] projection chunk: out_sb[:, ch, g*512:...] =
                W_ch.T @ xT[:, g-block] + b (drained on DVE). w0 selects the
                K (0) or Q (E) half of the fused wkq weights."""
                ssl = slice(g * 512, (g + 1) * 512)
                ps = ps_misc.tile([128, 512], F32, tag="misc")
                for k in range(KD):
                    nc.tensor.matmul(
                        ps[:],
                        wkq_sb[:, k, w0 + ch * 128:w0 + (ch + 1) * 128],
                        xT_sb[:, k, ssl],
                        start=(k == 0), stop=(k == KD - 1))
                nc.vector.tensor_scalar(
                    out_sb[:, ch, ssl], ps[:], b_sb[:, ch:ch + 1], None, ADD)

            def kproj(ch, g):
                proj_chain(0, bk_sb, KT_sb, ch, g)

            def qproj(qc, ch):
                proj_chain(E, bq_sb, QT_sb, ch, qc)

            def vproj(st):
                """V for s-tile st (all 4 heads), scattered+biased into the
                augmented V tiles on Pool."""
                ps = ps_misc.tile([128, 512], F32, tag="misc")
                for k in range(KD):
                    nc.tensor.matmul(
                        ps[:, 0:E],
                        xT_sb[:, k, st * 128:(st + 1) * 128],
                        wv_sb[:, k, :],
                        start=(k == 0), stop=(k == KD - 1))
                for h in range(4):
                    pr, odd = h // 2, h % 2
                    dst = (Vo_sb[pr][:, st, 64:128] if odd
                           else Ve_sb[pr][:, st, 0:64])
                    nc.vector.tensor_tensor(
                        dst, ps[:, h * 64:(h + 1) * 64],
                        bvb_sb[:, h * 64:(h + 1) * 64], ADD)

            def outproj_unit(st, nch):
                """y[s-tile st, nch*512:...] = OT[:, :, ssl].T @ woT (both
                contraction chunks), drained to fp16 on Pool, DMA'd on sync."""
                ssl = slice(st * 128, (st + 1) * 128)
                psy = ps_misc.tile([128, 512], F32, tag="misc")
                for cc in range(2):
                    nc.tensor.matmul(
                        psy[:], OT_sb[:, cc, ssl],
                        woT_sb[:, cc, nch * 512:(nch + 1) * 512],
                        start=(cc == 0), stop=(cc == 1))
                y_sb = yp.tile([128, 512], F16, tag="y")
                nc.vector.tensor_copy(y_sb[:], psy[:])
                eng = nc.sync if nch == 0 else nc.gpsimd
                eng.dma_start(y[ssl, nch * 512:(nch + 1) * 512], y_sb[:])

            def attn_iter(qc, pr, fills):
                """One (q-chunk, head-pair) attention iteration. `fills` maps
                kt -> list of emit callables sprinkled into the loop to keep
                the PE stream dense. AV matmuls run AV_LAG k-tiles behind the
                score matmuls so the PE never waits on the exp latency.
                Returns a callable that emits the PE/DVE/Pool back half of the
                normalization (scheduled as a fill in the next iteration)."""
                qsl = slice(qc * QC, (qc + 1) * QC)
                av_e = ps_av.tile([128, QC], F32, tag="av")
                av_o = ps_av.tile([128, QC], F32, tag="av")
                pexps = {}
                AV_LAG = 3

                def emit_av(kt):
                    pexp = pexps.pop(kt)
                    nc.tensor.matmul(av_e[:], Ve_sb[pr][:, kt, :],
                                     pexp[:, 0, :],
                                     start=(kt == 0), stop=(kt == N_KT - 1))
                    nc.tensor.matmul(av_o[:], Vo_sb[pr][:, kt, :],
                                     pexp[:, 1, :],
                                     start=(kt == 0), stop=(kt == N_KT - 1))

                for kt in range(N_KT):
                    for f in fills.get(kt, ()):
                        f()
                    ksl = slice(kt * 128, (kt + 1) * 128)
                    # scores for the head pair: two K=64 matmuls on PE row
                    # groups 0:63 / 64:127, issued back-to-back so they run
                    # concurrently on the array.
                    ps = ps_s.tile([128, 2, QC], F32, tag="s")
                    nc.tensor.matmul(ps[:, 0, :], KT_sb[0:64, pr, ksl],
                                     QT_sb[0:64, pr, qsl])
                    nc.tensor.matmul(ps[:, 1, :], KT_sb[64:128, pr, ksl],
                                     QT_sb[64:128, pr, qsl])
                    pexp = pxp.tile([128, 2, QC], F16, tag="pexp")
                    pexps[kt] = pexp
                    if qc > 0 and kt in OFF_DVE:
                        nc.vector.tensor_scalar(
                            pexp[:].bitcast(I16), ps[:], SCHR_A, SCHR_B,
                            MULT, ADD)
                    else:
                        nc.scalar.activation(pexp[:], ps[:], EXP, scale=0.125)
                    if kt >= AV_LAG:
                        emit_av(kt - AV_LAG)
                for kt in range(N_KT - AV_LAG, N_KT):
                    emit_av(kt)
                # normalization, front half (DVE): sums rows to SBUF fp16
                # first (unblocks the broadcast matmuls), then drain psav to
                # SBUF (frees the PSUM banks for the next iteration).
                avsb_e = wk.tile([128, QC], F32, tag="avsb")
                avsb_o = wk.tile([128, QC], F32, tag="avsb")
                sums16 = wk.tile([128, QC], F16, tag="sums16")
                nc.vector.tensor_copy(sums16[64:65, :], av_e[64:65, :])
                nc.vector.tensor_copy(sums16[0:1, :], av_o[0:1, :])
                nc.vector.tensor_copy(avsb_e[:], av_e[:])
                nc.vector.tensor_copy(avsb_o[:], av_o[:])

                def finish_norm(fast=False):
                    # broadcast the raw sums (at partition 64 for even / 0
                    # for odd) via two K=1 ones-matmuls into one PSUM tile,
                    # one approx reciprocal over all 128 partitions, then
                    # scale on Pool (all-SBUF operands).
                    psbc = ps_misc.tile([128, 512], F32, tag="misc")
                    nc.tensor.matmul(psbc[0:64, :], ones_sb[64:65, 0:64],
                                     sums16[64:65, :])
                    nc.tensor.matmul(psbc[64:128, :], ones_sb[0:1, 64:128],
                                     sums16[0:1, :])
                    rec = rp.tile([128, QC], F32, tag="rec")
                    nc.vector.reciprocal_approx_fast(rec[:], psbc[:])
                    eng = nc.vector if (fast or pr == 1) else nc.gpsimd
                    eng.tensor_tensor(
                        OT_sb[0:64, pr, qsl], avsb_e[0:64, :], rec[0:64, :],
                        MULT)
                    eng.tensor_tensor(
                        OT_sb[64:128, pr, qsl], avsb_o[64:128, :],
                        rec[64:128, :], MULT)

                return finish_norm

            # ---------------- fused main loop ----------------
            # Warmup: K/Q first (their weights lead the scalar queue; scores
            # can then start early, warming up the scalar engine), V after
            # (its weights stream in on the gpsimd queue meanwhile).
            kproj(0, 0)
            qproj(0, 0)
            vproj(0)
            vproj(1)
            kproj(0, 1)
            vproj(2)
            vproj(3)
            vproj(4)
            vproj(5)

            pending_norm = None
            for qc in range(N_QC):
                for pr in range(2):
                    fills = {}
                    if pending_norm is not None:
                        fills.setdefault(1, []).append(pending_norm)
                    if qc == 0:
                        # JIT K/V projections. K chunk `pr` group g must
                        # precede scores kt=4g; V tile st must precede
                        # AV kt=st (prefetch distance 2 + AV lag).
                        if pr == 0:
                            for g in range(2, 4):
                                fills.setdefault(4 * (g - 1) + 2, []).append(
                                    (lambda g=g: kproj(0, g)))
                            for st in range(6, N_KT):
                                fills.setdefault(st - 6, []).append(
                                    (lambda st=st: vproj(st)))
                            fills.setdefault(12, []).append(lambda: qproj(0, 1))
                            fills.setdefault(13, []).append(lambda: kproj(1, 0))
                        else:
                            for g in range(1, 4):
                                fills.setdefault(4 * (g - 1) + 2, []).append(
                                    (lambda g=g: kproj(1, g)))
                            fills.setdefault(12, []).append(lambda: qproj(1, 0))
                    else:
                        # out-projection of the previous q-chunk: 8 units
                        # spread across the two pr iterations.
                        for i, slot in enumerate((6, 9, 12, 15)):
                            u = pr * 4 + i
                            st, nch = (qc - 1) * 4 + u // 2, u % 2
                            fills.setdefault(slot, []).append(
                                (lambda st=st, nch=nch: outproj_unit(st, nch)))
                        if pr == 0:
                            fills.setdefault(12, []).append(
                                (lambda qc=qc: qproj(qc, 1)))
                        elif qc < N_QC - 1:
                            fills.setdefault(12, []).append(
                                (lambda qc=qc: qproj(qc + 1, 0)))
                    pending_norm = attn_iter(qc, pr, fills)
            pending_norm(fast=True)

            # tail: out-projection of the last q-chunk
            for u in range(8):
                st, nch = (N_QC - 1) * 4 + u // 2, u % 2
                outproj_unit(st, nch)

    nc.compile()
    return nc


_NC_CACHE = None
last_in_maps = None


def kernel(x, Wq, bq, Wk, bk, Wv, bv, Wo, bo):
    global _NC_CACHE, last_in_maps
    x = np.asarray(x, dtype=np.float32)
    Wq, bq = np.asarray(Wq, np.float32), np.asarray(bq, np.float32)
    Wk, bk = np.asarray(Wk, np.float32), np.asarray(bk, np.float32)
    Wv, bv = np.asarray(Wv, np.float32), np.asarray(bv, np.float32)
    Wo, bo = np.asarray(Wo, np.float32), np.asarray(bo, np.float32)

    if _NC_CACHE is None:
        _NC_CACHE = build()
    nc = _NC_CACHE

    vce = np.zeros((128, N_KT, 128), np.float16)
    vce[:, :, 64] = 1.0
    vco = np.zeros((128, N_KT, 128), np.float16)
    vco[:, :, 0] = 1.0

    in_maps = []
    for c in range(8):
        b, f0 = c // 4, (c % 4) * E
        fs = slice(f0, f0 + E)
        in_maps.append(dict(
            xT=np.ascontiguousarray(x[b].T).astype(np.float16),
            wkq=np.ascontiguousarray(np.concatenate(
                [Wk[fs, :].T, Wq[fs, :].T], axis=1)).astype(np.float16),
            wvT=np.ascontiguousarray(Wv[fs, :].T).astype(np.float16),
            woT=np.ascontiguousarray(Wo[:, fs].T).astype(np.float16),
            bq2=np.ascontiguousarray(bq[fs].reshape(2, 128).T),
            bk2=np.ascontiguousarray(bk[fs].reshape(2, 128).T),
            bvb=np.ascontiguousarray(np.broadcast_to(bv[fs], (128, E))),
            vce=vce,
            vco=vco,
        ))

    last_in_maps = in_maps
    res = bass_utils.run_bass_kernel_spmd(nc, in_maps, core_ids=list(range(8)))

    out = np.zeros((B, S, D), np.float32)
    for c in range(8):
        out[c // 4] += res.results[c]["y"].astype(np.float32)
    out += bo
    return out
